# revision 29
# baseline (speedup 1.0000x reference)
"""Trainium2 Bass kernel for MinEuclideanDistBlock.

Math (per batch b):
  d2[c,w,k] = ||x[b,c,w:w+S]||^2 + ||sh[c,k]||^2 - 2 <x[b,c,w:w+S], sh[c,k]>
  out[b,k]  = min_w  sum_c sqrt(d2[c,w,k])

Kernel strategy (per core, data-parallel over batch: 16 of 128 batches):
  - One matmul per (b,c,phase) produces d2 directly in PSUM via an
    augmented 61-row contraction: 56 rows of a stride-4 im2col of x
    (phase-major groups of 14), 4 rows of the phase-split sliding
    ||window||^2 (computed once by a log-doubling shift-add tree), and a
    ones row paired with sq_s.
  - The stride-4 im2col cuts duplication ~4x vs stride-1; its source
    (the phase-deinterleaved x) is round-tripped through HBM so the
    per-(b,c) im2col read is a DRAM->SBUF transfer of contiguous 1KB
    lines (SBUF->SBUF overlapping-line DMA measured ~3GB/s/engine on
    the previous version; DRAM reads are the fast standard pattern).
  - d2 is phase-major [K, 4*512]; column order is irrelevant under the
    final min over windows, so each matmul writes one contiguous PSUM
    bank.  Out-of-range windows see a +50000 sq_w pad and become huge
    valid distances, so no masking is needed downstream.
  - dist = sqrt(d2): one scalar-engine activation per (b,c); this is the
    critical engine (~2us per (b,c) at 1 elem/cycle/lane).
  - channel sum + min over windows: two tensor_adds plus a min
    tensor_reduce per batch on the vector engine (tensor_tensor_reduce
    would fuse these but crashes the runtime on this stack).
  - matmul operands are bfloat16, sqw tree and dist tiles float16
    (end-to-end rel err ~2.4e-3, far inside the 2e-2 gate).
  - the PE clock is pinned at 1.2GHz in this environment (a sustained
    dummy-matmul burst does not flip the HAM clock gate), so the PE
    stream time is the binding engine alongside the scalar-engine sqrt.
"""

import numpy as np
from contextlib import ExitStack

import concourse.bass as bass
import concourse.bacc as bacc
import concourse.mybir as mybir
import concourse.tile as tile
from concourse import masks
from concourse.bass_utils import run_bass_kernel_spmd

B, C, L = 128, 3, 2048
K, S = 128, 50
W = L - S + 1  # 1999
NCORES = 8
BPC = B // NCORES  # batches per core
BC = BPC * C  # x rows per core

PH = 4  # im2col stride / number of window phases
T = 512  # columns per phase (= one PSUM bank of fp32)
RXP = 14  # im2col rows per phase group
NXROW = PH * RXP  # 56 x rows
CONTRACT = NXROW + PH + 1  # + sqw phase rows + ones(sq_s) row = 61
LQ = L // PH  # real elements per phase block (512)
BLK = LQ + RXP  # x4 phase-block pitch (526); max read 13+511=524 < 526
SQWPAD = 50000.0  # sq_w pad: any window w >= W gets d2 ~ 5e4 -> dist ~ 224

F32 = mybir.dt.float32
F16 = mybir.dt.float16
BF16 = mybir.dt.bfloat16
ACT = mybir.ActivationFunctionType
ALU = mybir.AluOpType
AXIS = mybir.AxisListType

LAST_RESULTS = None  # BassKernelResults of the last run (for test harness)


def _body(ctx, tc, out_ap, x_ap, sh_ap, xd_ap, sqd_ap):
    nc = tc.nc

    const = ctx.enter_context(tc.tile_pool(name="const", bufs=1))
    ident = const.tile([128, 128], F32)
    masks.make_identity(nc, ident[:])
    ones_blk = const.tile([1, C * K], BF16)
    nc.vector.memset(ones_blk[:], 1.0)
    # one stationary tile per phase, all 3 channels side by side in the free dim
    lhsT = [
        const.tile([CONTRACT, C * K], BF16, tag=f"lhsT{j}", name=f"lhsT{j}")
        for j in range(PH)
    ]
    persist = ctx.enter_context(tc.tile_pool(name="persist", bufs=1))
    res = persist.tile([K, BPC], F32)

    # ---- x load + x^2 first: ACT is idle this early, and the DVE tree
    # depends on xsq, so this pulls the whole x chain forward ----
    xp = ctx.enter_context(tc.tile_pool(name="xprep", bufs=1))
    x_all = xp.tile([BC, L], F32)
    x_rows = x_ap.rearrange("b c l -> (b c) l")
    for lo, hi, eng in (
        (0, 688, nc.sync),
        (688, 1376, nc.scalar),
        (1376, L, nc.gpsimd),
    ):
        eng.dma_start(x_all[:, lo:hi], x_rows[:, lo:hi])
    # x^2 in two column pieces so the early tree chain can start before the
    # whole x transfer lands
    XCUT = 1032
    xsq = xp.tile([BC, L], F16)
    nc.scalar.activation(xsq[:, :XCUT], x_all[:, :XCUT], ACT.Square)
    nc.scalar.activation(xsq[:, XCUT:], x_all[:, XCUT:], ACT.Square)

    # ---- shapelet prep: st4 (phase-major -2*sh^T | sq_s) and lhsT tiles ----
    # All prep DMAs go through the Activation DGE queue so the Sync queue is
    # free to start the main-loop im2col reads as soon as deps resolve.
    prep = ctx.enter_context(tc.tile_pool(name="prep", bufs=1))
    with tc.tile_pool(name="prep_ps", bufs=1, space="PSUM") as prep_ps:
        pt = prep_ps.tile([53, C * K], F32, tag="pt")
        st4 = prep.tile([53, C * K], BF16, tag="st4")
        for c in range(C):
            sh_pad = prep.tile([K, 52], F32, tag=f"sh_pad{c}")
            nc.vector.memset(sh_pad[:, S:], 0.0)
            nc.scalar.dma_start(sh_pad[:, :S], sh_ap[c])
            # sh_m2p cols q = 13*(s%4) + s//4 hold -2*sh[:, s]; col 52 = sq_s
            sh_m2p = prep.tile([K, 53], F32, tag=f"sh_m2p{c}")
            perm_dst = bass.AP(
                sh_m2p.tensor, sh_m2p.offset, [[sh_m2p.ap[0][0], K], [13, PH], [1, 13]]
            )
            perm_src = bass.AP(
                sh_pad.tensor, sh_pad.offset, [[sh_pad.ap[0][0], K], [1, PH], [PH, 13]]
            )
            nc.scalar.activation(perm_dst, perm_src, ACT.Copy, scale=-2.0)
            sh_sq = prep.tile([K, S], F32, tag=f"sh_sq{c}")
            nc.scalar.activation(
                sh_sq[:], sh_pad[:, :S], ACT.Square,
                accum_out=sh_m2p[:, 52:53],
            )
            nc.tensor.transpose(pt[:, c * K : (c + 1) * K], sh_m2p[:], ident[:])
        nc.scalar.activation(st4[:], pt[:], ACT.Copy)
        # touch Sqrt now so its ACT table loads during prep, not at the
        # first main-loop tile
        sq_warm = prep.tile([1, 1], F32, tag="sq_warm")
        nc.scalar.activation(sq_warm[:], ident[0:1, 0:1], ACT.Sqrt)
        for j in range(PH):
            lt = lhsT[j]
            nc.gpsimd.memset(lt[:], 0.0)
            for p in range(PH):
                d = (p - j) % PH
                a0 = 0 if p >= j else 1
                eng = nc.scalar if p % 2 == 0 else nc.sync
                eng.dma_start(
                    lt[p * RXP + a0 : p * RXP + a0 + 13, :],
                    st4[d * 13 : d * 13 + 13, :],
                )
            # sqw coefficient: phase j row
            nc.scalar.dma_start(lt[NXROW + j : NXROW + j + 1, :], ones_blk[:])
            # sq_s row (pairs with the ones block of sqw4)
            nc.sync.dma_start(lt[CONTRACT - 1 : CONTRACT, :], st4[52:53, :])

    # ---- x prep: deinterleave, squares tree, spill to DRAM ----
    if True:
        # x4[bc, p*BLK + z] = x[bc, PH*z + p] for z < LQ, 0.0 in the pad tail
        x4 = xp.tile([BC, PH * BLK], BF16)
        pad = bass.AP(
            x4.tensor,
            x4[0:1, LQ : LQ + 1].offset,
            [[x4.ap[0][0], BC], [BLK, PH], [1, BLK - LQ]],
        )
        nc.gpsimd.memset(pad, 0.0)
        x_deint = bass.AP(
            x_all.tensor, x_all.offset, [[x_all.ap[0][0], BC], [1, PH], [PH, LQ]]
        )
        x4_dst = bass.AP(
            x4.tensor, x4.offset, [[x4.ap[0][0], BC], [BLK, PH], [1, LQ]]
        )
        s2 = xp.tile([BC, L], F16)
        s4 = xp.tile([BC, L], F16)
        s8 = xp.tile([BC, L], F16)
        s16 = xp.tile([BC, L], F16)
        s32 = xp.tile([BC, L], F16)
        s48 = xp.tile([BC, L], F16)
        sqw = xp.tile([BC, W + 1], F16)
        nc.gpsimd.memset(sqw[:, W : W + 1], SQWPAD)
        levels = [
            (s2, xsq, xsq, 1, L - 1, 1024),
            (s4, s2, s2, 2, L - 3, 1022),
            (s8, s4, s4, 4, L - 7, 1018),
            (s16, s8, s8, 8, L - 15, 1010),
            (s32, s16, s16, 16, L - 31, 994),
            (s48, s32, s16, 32, L - 47, 962),
            (sqw, s48, s2, 48, W, 962),
        ]
        for dst, a, b, h, v, e in levels:  # early chain: only needs xsq[:XCUT]
            nc.vector.tensor_add(dst[:, :e], a[:, :e], b[:, h : h + e])
        nc.vector.tensor_copy(x4_dst, x_deint)
        nc.sync.dma_start(xd_ap, x4[:])
        for dst, a, b, h, v, e in levels:  # late chain
            nc.vector.tensor_add(dst[:, e:v], a[:, e:v], b[:, e + h : v + h])

        sqw4 = xp.tile([BC, (PH + 1) * T], BF16)
        nc.gpsimd.memset(sqw4[:], SQWPAD)
        ones_t = bass.AP(
            sqw4.tensor,
            sqw4[0:1, PH * T : PH * T + 1].offset,
            [[sqw4.ap[0][0], BC], [1, T]],
        )
        nc.gpsimd.memset(ones_t, 1.0)
        nq = 500  # deinterleave reads sqw cols j + PH*t, t < 500 (max 1999)
        sq_src = bass.AP(
            sqw.tensor, sqw.offset, [[sqw.ap[0][0], BC], [1, PH], [PH, nq]]
        )
        sq_dst = bass.AP(
            sqw4.tensor, sqw4.offset, [[sqw4.ap[0][0], BC], [T, PH], [1, nq]]
        )
        nc.vector.tensor_copy(sq_dst, sq_src)
        nc.gpsimd.dma_start(sqd_ap, sqw4[:])

    # ---- main loop ----
    rhsp = ctx.enter_context(tc.tile_pool(name="rhs", bufs=6))
    psum = ctx.enter_context(tc.tile_pool(name="mm", bufs=2, space="PSUM"))
    distp = ctx.enter_context(tc.tile_pool(name="dist", bufs=4))

    for b in range(BPC):
        dist = []
        for c in range(C):
            bc = b * C + c
            rhs = rhsp.tile([CONTRACT, T], BF16, tag="rhs")
            # x im2col rows from DRAM: row p*RXP+a = x4[bc, p*BLK + a + t]
            nc.sync.dma_start(
                rhs[:NXROW, :],
                bass.AP(
                    xd_ap.tensor,
                    bc * PH * BLK,
                    [[PH * BLK, 1], [BLK, PH], [1, RXP], [1, T]],
                ),
            )
            # sqw phase rows + ones row (issued from GpSimd's queue to keep
            # the Sync queue for the big im2col reads).  The first batches
            # read straight from the SBUF sqw4 tile so they need not wait
            # for the DRAM spill round-trip; later batches read DRAM, whose
            # contiguous lines are the fast pattern at steady state.
            if b < 2:
                sq_src = bass.AP(
                    sqw4.tensor,
                    sqw4[bc : bc + 1, 0:1].offset,
                    [[sqw4.ap[0][0], 1], [T, PH + 1], [1, T]],
                )
            else:
                sq_src = bass.AP(
                    sqd_ap.tensor,
                    bc * (PH + 1) * T,
                    [[(PH + 1) * T, 1], [T, PH + 1], [1, T]],
                )
            nc.gpsimd.dma_start(rhs[NXROW:CONTRACT, :], sq_src)
            d2 = psum.tile([K, PH * T], F32, tag="d2")
            for j in range(PH):
                nc.tensor.matmul(
                    d2[:, j * T : (j + 1) * T],
                    lhsT[j][:, c * K : (c + 1) * K],
                    rhs[:],
                    start=True,
                    stop=True,
                )
            dt_ = distp.tile([K, PH * T], F16, tag=f"dist{c}", name=f"dist{c}")
            nc.scalar.activation(dt_[:], d2[:], ACT.Sqrt)
            dist.append(dt_)
        t01 = distp.tile([K, PH * T], F16, tag="t01")
        nc.vector.tensor_add(t01[:], dist[0][:], dist[1][:])
        scr = distp.tile([K, PH * T], F16, tag="scr")
        nc.vector.tensor_add(scr[:], t01[:], dist[2][:])
        nc.vector.tensor_reduce(
            res[:, b : b + 1], scr[:], axis=AXIS.X, op=ALU.min
        )

    # ---- store result as (K, BPC); the host unshard transposes ----
    nc.sync.dma_start(out_ap, res[:])


def _build():
    nc = bacc.Bacc(
        "TRN2", target_bir_lowering=False, debug=False, num_devices=NCORES
    )
    x = nc.dram_tensor("x", [BPC, C, L], F32, kind="ExternalInput").ap()
    sh = nc.dram_tensor("sh", [C, K, S], F32, kind="ExternalInput").ap()
    out = nc.dram_tensor("out", [K, BPC], F32, kind="ExternalOutput").ap()
    xd = nc.dram_tensor("xs4", [BC, PH * BLK], BF16, kind="Internal").ap()
    sqd = nc.dram_tensor("sqw4", [BC, (PH + 1) * T], BF16, kind="Internal").ap()
    with tile.TileContext(nc) as tc, ExitStack() as ctx:
        _body(ctx, tc, out, x, sh, xd, sqd)
    nc.compile()
    return nc


def kernel(x, shapelets, trace=False):
    global LAST_RESULTS
    x = np.ascontiguousarray(np.asarray(x, dtype=np.float32))
    shapelets = np.ascontiguousarray(np.asarray(shapelets, dtype=np.float32))
    nc = _build()
    in_maps = [
        {"x": x[i * BPC : (i + 1) * BPC], "sh": shapelets} for i in range(NCORES)
    ]
    results = run_bass_kernel_spmd(
        nc, in_maps, core_ids=list(range(NCORES)), trace=trace
    )
    LAST_RESULTS = results
    out = np.concatenate(
        [results.results[i]["out"].T for i in range(NCORES)], axis=0
    )
    return np.ascontiguousarray(out).reshape(B, 1, K)


# revision 31
# speedup vs baseline: 1.1478x; 1.1478x over previous
"""Trainium2 Bass kernel for MinEuclideanDistBlock.

Math (per batch b):
  d2[c,w,k] = ||x[b,c,w:w+S]||^2 + ||sh[c,k]||^2 - 2 <x[b,c,w:w+S], sh[c,k]>
  out[b,k]  = min_w  sum_c sqrt(d2[c,w,k])

Kernel strategy (per core, data-parallel over batch: 16 of 128 batches):
  - One matmul per (b,c,phase) produces d2 directly in PSUM via an
    augmented 61-row contraction: 56 rows of a stride-4 im2col of x
    (phase-major groups of 14), 4 rows of the phase-split sliding
    ||window||^2 (computed once by a log-doubling shift-add tree), and a
    ones row paired with sq_s.
  - The stride-4 im2col cuts duplication ~4x vs stride-1; its source
    (the phase-deinterleaved x) is round-tripped through HBM so the
    per-(b,c) im2col read is a DRAM->SBUF transfer of contiguous 1KB
    lines (SBUF->SBUF overlapping-line DMA measured ~3GB/s/engine on
    the previous version; DRAM reads are the fast standard pattern).
  - d2 is phase-major [K, 4*512]; column order is irrelevant under the
    final min over windows, so each matmul writes one contiguous PSUM
    bank.  Out-of-range windows see a +50000 sq_w pad and become huge
    valid distances, so no masking is needed downstream.
  - dist = sqrt(d2): one scalar-engine activation per (b,c); this is the
    critical engine (~2us per (b,c) at 1 elem/cycle/lane).
  - channel sum + min over windows: two tensor_adds plus a min
    tensor_reduce per batch on the vector engine (tensor_tensor_reduce
    would fuse these but crashes the runtime on this stack).
  - matmul operands are bfloat16, sqw tree and dist tiles float16
    (end-to-end rel err ~2.4e-3, far inside the 2e-2 gate).
  - the PE clock is pinned at 1.2GHz in this environment (a sustained
    dummy-matmul burst does not flip the HAM clock gate), so the PE
    stream time is the binding engine alongside the scalar-engine sqrt.
"""

import numpy as np
from contextlib import ExitStack

import concourse.bass as bass
import concourse.bacc as bacc
import concourse.mybir as mybir
import concourse.tile as tile
from concourse import masks
from concourse.bass_utils import run_bass_kernel_spmd

B, C, L = 128, 3, 2048
K, S = 128, 50
W = L - S + 1  # 1999
NCORES = 8
BPC = B // NCORES  # batches per core
BC = BPC * C  # x rows per core

PH = 4  # im2col stride / number of window phases
T = 512  # columns per phase (= one PSUM bank of fp32)
RXP = 14  # im2col rows per phase group
NXROW = PH * RXP  # 56 x rows
CONTRACT = NXROW + PH + 1  # + sqw phase rows + ones(sq_s) row = 61
LQ = L // PH  # real elements per phase block (512)
BLK = LQ + RXP  # x4 phase-block pitch (526); max read 13+511=524 < 526
SQWPAD = 50000.0  # sq_w pad: any window w >= W gets d2 ~ 5e4 -> dist ~ 224

F32 = mybir.dt.float32
F16 = mybir.dt.float16
BF16 = mybir.dt.bfloat16
ACT = mybir.ActivationFunctionType
ALU = mybir.AluOpType
AXIS = mybir.AxisListType

LAST_RESULTS = None  # BassKernelResults of the last run (for test harness)


def _body(ctx, tc, out_ap, x_ap, sh_ap, xd_ap, sqd_ap):
    nc = tc.nc

    const = ctx.enter_context(tc.tile_pool(name="const", bufs=1))
    ident = const.tile([128, 128], F32)
    masks.make_identity(nc, ident[:])
    ones_blk = const.tile([1, C * K], BF16)
    nc.vector.memset(ones_blk[:], 1.0)
    # one stationary tile per phase, all 3 channels side by side in the free dim
    lhsT = [
        const.tile([CONTRACT, C * K], BF16, tag=f"lhsT{j}", name=f"lhsT{j}")
        for j in range(PH)
    ]
    persist = ctx.enter_context(tc.tile_pool(name="persist", bufs=1))
    res = persist.tile([K, BPC], F32)

    # ---- x load + x^2 first: ACT is idle this early, and the DVE tree
    # depends on xsq, so this pulls the whole x chain forward ----
    xp = ctx.enter_context(tc.tile_pool(name="xprep", bufs=1))
    x_all = xp.tile([BC, L], F32)
    x_rows = x_ap.rearrange("b c l -> (b c) l")
    for lo, hi, eng in (
        (0, 688, nc.sync),
        (688, 1376, nc.scalar),
        (1376, L, nc.gpsimd),
    ):
        eng.dma_start(x_all[:, lo:hi], x_rows[:, lo:hi])
    # x^2 in two column pieces so the early tree chain can start before the
    # whole x transfer lands
    XCUT = 1032
    xsq = xp.tile([BC, L], F16)
    nc.scalar.activation(xsq[:, :XCUT], x_all[:, :XCUT], ACT.Square)
    nc.scalar.activation(xsq[:, XCUT:], x_all[:, XCUT:], ACT.Square)

    # ---- shapelet prep: st4 (phase-major -2*sh^T | sq_s) and lhsT tiles ----
    # All prep DMAs go through the Activation DGE queue so the Sync queue is
    # free to start the main-loop im2col reads as soon as deps resolve.
    prep = ctx.enter_context(tc.tile_pool(name="prep", bufs=1))
    with tc.tile_pool(name="prep_ps", bufs=1, space="PSUM") as prep_ps:
        pt = prep_ps.tile([53, C * K], F32, tag="pt")
        st4 = prep.tile([53, C * K], BF16, tag="st4")
        for c in range(C):
            sh_pad = prep.tile([K, 52], F32, tag=f"sh_pad{c}")
            nc.vector.memset(sh_pad[:, S:], 0.0)
            nc.scalar.dma_start(sh_pad[:, :S], sh_ap[c])
            # sh_m2p cols q = 13*(s%4) + s//4 hold -2*sh[:, s]; col 52 = sq_s
            sh_m2p = prep.tile([K, 53], F32, tag=f"sh_m2p{c}")
            perm_dst = bass.AP(
                sh_m2p.tensor, sh_m2p.offset, [[sh_m2p.ap[0][0], K], [13, PH], [1, 13]]
            )
            perm_src = bass.AP(
                sh_pad.tensor, sh_pad.offset, [[sh_pad.ap[0][0], K], [1, PH], [PH, 13]]
            )
            nc.scalar.activation(perm_dst, perm_src, ACT.Copy, scale=-2.0)
            sh_sq = prep.tile([K, S], F32, tag=f"sh_sq{c}")
            nc.scalar.activation(
                sh_sq[:], sh_pad[:, :S], ACT.Square,
                accum_out=sh_m2p[:, 52:53],
            )
            nc.tensor.transpose(pt[:, c * K : (c + 1) * K], sh_m2p[:], ident[:])
        nc.scalar.activation(st4[:], pt[:], ACT.Copy)
        # touch Sqrt now so its ACT table loads during prep, not at the
        # first main-loop tile
        sq_warm = prep.tile([1, 1], F32, tag="sq_warm")
        nc.scalar.activation(sq_warm[:], ident[0:1, 0:1], ACT.Sqrt)
        for j in range(PH):
            lt = lhsT[j]
            nc.gpsimd.memset(lt[:], 0.0)
            for p in range(PH):
                d = (p - j) % PH
                a0 = 0 if p >= j else 1
                eng = nc.scalar if p % 2 == 0 else nc.sync
                eng.dma_start(
                    lt[p * RXP + a0 : p * RXP + a0 + 13, :],
                    st4[d * 13 : d * 13 + 13, :],
                )
            # sqw coefficient: phase j row
            nc.scalar.dma_start(lt[NXROW + j : NXROW + j + 1, :], ones_blk[:])
            # sq_s row (pairs with the ones block of sqw4)
            nc.sync.dma_start(lt[CONTRACT - 1 : CONTRACT, :], st4[52:53, :])

    # ---- x prep: deinterleave, squares tree, spill to DRAM ----
    if True:
        # x4[bc, p*BLK + z] = x[bc, PH*z + p] for z < LQ, 0.0 in the pad tail
        x4 = xp.tile([BC, PH * BLK], BF16)
        pad = bass.AP(
            x4.tensor,
            x4[0:1, LQ : LQ + 1].offset,
            [[x4.ap[0][0], BC], [BLK, PH], [1, BLK - LQ]],
        )
        nc.gpsimd.memset(pad, 0.0)
        x_deint = bass.AP(
            x_all.tensor, x_all.offset, [[x_all.ap[0][0], BC], [1, PH], [PH, LQ]]
        )
        x4_dst = bass.AP(
            x4.tensor, x4.offset, [[x4.ap[0][0], BC], [BLK, PH], [1, LQ]]
        )
        s2 = xp.tile([BC, L], F16)
        s4 = xp.tile([BC, L], F16)
        s8 = xp.tile([BC, L], F16)
        s16 = xp.tile([BC, L], F16)
        s32 = xp.tile([BC, L], F16)
        s48 = xp.tile([BC, L], F16)
        sqw = xp.tile([BC, W + 1], F16)
        nc.gpsimd.memset(sqw[:, W : W + 1], SQWPAD)
        levels = [
            (s2, xsq, xsq, 1, L - 1, 1024),
            (s4, s2, s2, 2, L - 3, 1022),
            (s8, s4, s4, 4, L - 7, 1018),
            (s16, s8, s8, 8, L - 15, 1010),
            (s32, s16, s16, 16, L - 31, 994),
            (s48, s32, s16, 32, L - 47, 962),
            (sqw, s48, s2, 48, W, 962),
        ]
        for dst, a, b, h, v, e in levels:  # early chain: only needs xsq[:XCUT]
            nc.vector.tensor_add(dst[:, :e], a[:, :e], b[:, h : h + e])
        nc.vector.tensor_copy(x4_dst, x_deint)
        nc.sync.dma_start(xd_ap, x4[:])
        for dst, a, b, h, v, e in levels:  # late chain
            nc.vector.tensor_add(dst[:, e:v], a[:, e:v], b[:, e + h : v + h])

        sqw4 = xp.tile([BC, (PH + 1) * T], BF16)
        nc.gpsimd.memset(sqw4[:], SQWPAD)
        ones_t = bass.AP(
            sqw4.tensor,
            sqw4[0:1, PH * T : PH * T + 1].offset,
            [[sqw4.ap[0][0], BC], [1, T]],
        )
        nc.gpsimd.memset(ones_t, 1.0)
        nq = 500  # deinterleave reads sqw cols j + PH*t, t < 500 (max 1999)
        sq_src = bass.AP(
            sqw.tensor, sqw.offset, [[sqw.ap[0][0], BC], [1, PH], [PH, nq]]
        )
        sq_dst = bass.AP(
            sqw4.tensor, sqw4.offset, [[sqw4.ap[0][0], BC], [T, PH], [1, nq]]
        )
        nc.vector.tensor_copy(sq_dst, sq_src)
        nc.gpsimd.dma_start(sqd_ap, sqw4[:])

    # ---- main loop ----
    rhsp = ctx.enter_context(tc.tile_pool(name="rhs", bufs=6))
    psum = ctx.enter_context(tc.tile_pool(name="mm", bufs=2, space="PSUM"))
    distp = ctx.enter_context(tc.tile_pool(name="dist", bufs=3))

    for b in range(BPC):
        dist = []
        for c in range(C):
            bc = b * C + c
            rhs = rhsp.tile([CONTRACT, T], BF16, tag="rhs")
            # x im2col rows from DRAM: row p*RXP+a = x4[bc, p*BLK + a + t]
            nc.sync.dma_start(
                rhs[:NXROW, :],
                bass.AP(
                    xd_ap.tensor,
                    bc * PH * BLK,
                    [[PH * BLK, 1], [BLK, PH], [1, RXP], [1, T]],
                ),
            )
            # sqw phase rows + ones row from DRAM.  Steady state issues from
            # GpSimd's queue (keeping Sync for the big im2col reads); the
            # first batches use Sync's hardware DGE, which has lower issue
            # latency than GpSimd's software descriptor generation.
            sq_eng = nc.sync if b < 3 else nc.gpsimd
            sq_eng.dma_start(
                rhs[NXROW:CONTRACT, :],
                bass.AP(
                    sqd_ap.tensor,
                    bc * (PH + 1) * T,
                    [[(PH + 1) * T, 1], [T, PH + 1], [1, T]],
                ),
            )
            d2 = psum.tile([K, PH * T], F32, tag="d2")
            for j in range(PH):
                nc.tensor.matmul(
                    d2[:, j * T : (j + 1) * T],
                    lhsT[j][:, c * K : (c + 1) * K],
                    rhs[:],
                    start=True,
                    stop=True,
                )
            dt_ = distp.tile([K, PH * T], F16, tag=f"dist{c}", name=f"dist{c}")
            nc.scalar.activation(dt_[:], d2[:], ACT.Sqrt)
            dist.append(dt_)
        t01 = distp.tile([K, PH * T], F16, tag="t01")
        nc.vector.tensor_add(t01[:], dist[0][:], dist[1][:])
        scr = distp.tile([K, PH * T], F16, tag="scr")
        nc.vector.tensor_add(scr[:], t01[:], dist[2][:])
        nc.vector.tensor_reduce(
            res[:, b : b + 1], scr[:], axis=AXIS.X, op=ALU.min
        )

    # ---- store result as (K, BPC); the host unshard transposes ----
    nc.sync.dma_start(out_ap, res[:])


def _build():
    nc = bacc.Bacc(
        "TRN2", target_bir_lowering=False, debug=False, num_devices=NCORES
    )
    x = nc.dram_tensor("x", [BPC, C, L], F32, kind="ExternalInput").ap()
    sh = nc.dram_tensor("sh", [C, K, S], F32, kind="ExternalInput").ap()
    out = nc.dram_tensor("out", [K, BPC], F32, kind="ExternalOutput").ap()
    xd = nc.dram_tensor("xs4", [BC, PH * BLK], BF16, kind="Internal").ap()
    sqd = nc.dram_tensor("sqw4", [BC, (PH + 1) * T], BF16, kind="Internal").ap()
    with tile.TileContext(nc) as tc, ExitStack() as ctx:
        _body(ctx, tc, out, x, sh, xd, sqd)
    nc.compile()
    return nc


def kernel(x, shapelets, trace=False):
    global LAST_RESULTS
    x = np.ascontiguousarray(np.asarray(x, dtype=np.float32))
    shapelets = np.ascontiguousarray(np.asarray(shapelets, dtype=np.float32))
    nc = _build()
    in_maps = [
        {"x": x[i * BPC : (i + 1) * BPC], "sh": shapelets} for i in range(NCORES)
    ]
    results = run_bass_kernel_spmd(
        nc, in_maps, core_ids=list(range(NCORES)), trace=trace
    )
    LAST_RESULTS = results
    out = np.concatenate(
        [results.results[i]["out"].T for i in range(NCORES)], axis=0
    )
    return np.ascontiguousarray(out).reshape(B, 1, K)


# revision 32
# speedup vs baseline: 1.1743x; 1.0231x over previous
"""Trainium2 Bass kernel for MinEuclideanDistBlock.

Math (per batch b):
  d2[c,w,k] = ||x[b,c,w:w+S]||^2 + ||sh[c,k]||^2 - 2 <x[b,c,w:w+S], sh[c,k]>
  out[b,k]  = min_w  sum_c sqrt(d2[c,w,k])

Kernel strategy (per core, data-parallel over batch: 16 of 128 batches):
  - One matmul per (b,c,phase) produces d2 directly in PSUM via an
    augmented 61-row contraction: 56 rows of a stride-4 im2col of x
    (phase-major groups of 14), 4 rows of the phase-split sliding
    ||window||^2 (computed once by a log-doubling shift-add tree), and a
    ones row paired with sq_s.
  - The stride-4 im2col cuts duplication ~4x vs stride-1; its source
    (the phase-deinterleaved x) is round-tripped through HBM so the
    per-(b,c) im2col read is a DRAM->SBUF transfer of contiguous 1KB
    lines (SBUF->SBUF overlapping-line DMA measured ~3GB/s/engine on
    the previous version; DRAM reads are the fast standard pattern).
  - d2 is phase-major [K, 4*512]; column order is irrelevant under the
    final min over windows, so each matmul writes one contiguous PSUM
    bank.  Out-of-range windows see a +50000 sq_w pad and become huge
    valid distances, so no masking is needed downstream.
  - dist = sqrt(d2): one scalar-engine activation per (b,c); this is the
    critical engine (~2us per (b,c) at 1 elem/cycle/lane).
  - channel sum + min over windows: two tensor_adds plus a min
    tensor_reduce per batch on the vector engine (tensor_tensor_reduce
    would fuse these but crashes the runtime on this stack).
  - matmul operands are bfloat16, sqw tree and dist tiles float16
    (end-to-end rel err ~2.4e-3, far inside the 2e-2 gate).
  - the PE clock is pinned at 1.2GHz in this environment (a sustained
    dummy-matmul burst does not flip the HAM clock gate), so the PE
    stream time is the binding engine alongside the scalar-engine sqrt.
"""

import numpy as np
from contextlib import ExitStack

import concourse.bass as bass
import concourse.bacc as bacc
import concourse.mybir as mybir
import concourse.tile as tile
from concourse import masks
from concourse.bass_utils import run_bass_kernel_spmd

B, C, L = 128, 3, 2048
K, S = 128, 50
W = L - S + 1  # 1999
NCORES = 8
BPC = B // NCORES  # batches per core
BC = BPC * C  # x rows per core

PH = 4  # im2col stride / number of window phases
T = 512  # columns per phase (= one PSUM bank of fp32)
RXP = 14  # im2col rows per phase group
NXROW = PH * RXP  # 56 x rows
CONTRACT = NXROW + PH + 1  # + sqw phase rows + ones(sq_s) row = 61
LQ = L // PH  # real elements per phase block (512)
BLK = LQ + RXP  # x4 phase-block pitch (526); max read 13+511=524 < 526
SQWPAD = 50000.0  # sq_w pad: any window w >= W gets d2 ~ 5e4 -> dist ~ 224

F32 = mybir.dt.float32
F16 = mybir.dt.float16
BF16 = mybir.dt.bfloat16
ACT = mybir.ActivationFunctionType
ALU = mybir.AluOpType
AXIS = mybir.AxisListType

LAST_RESULTS = None  # BassKernelResults of the last run (for test harness)


def _body(ctx, tc, out_ap, x_ap, sh_ap, xd_ap, sqd_ap):
    nc = tc.nc

    const = ctx.enter_context(tc.tile_pool(name="const", bufs=1))
    ident = const.tile([128, 128], F32)
    masks.make_identity(nc, ident[:])
    ones_blk = const.tile([1, C * K], BF16)
    nc.vector.memset(ones_blk[:], 1.0)
    # one stationary tile per phase, all 3 channels side by side in the free dim
    lhsT = [
        const.tile([CONTRACT, C * K], BF16, tag=f"lhsT{j}", name=f"lhsT{j}")
        for j in range(PH)
    ]
    persist = ctx.enter_context(tc.tile_pool(name="persist", bufs=1))
    res = persist.tile([K, BPC], F32)

    # ---- x load + x^2 first: ACT is idle this early, and the DVE tree
    # depends on xsq, so this pulls the whole x chain forward ----
    xp = ctx.enter_context(tc.tile_pool(name="xprep", bufs=1))
    x_all = xp.tile([BC, L], F32)
    x_rows = x_ap.rearrange("b c l -> (b c) l")
    for lo, hi, eng in (
        (0, 688, nc.sync),
        (688, 1376, nc.scalar),
        (1376, L, nc.gpsimd),
    ):
        eng.dma_start(x_all[:, lo:hi], x_rows[:, lo:hi])
    # x^2 in two column pieces so the early tree chain can start before the
    # whole x transfer lands
    XCUT = 1032
    xsq = xp.tile([BC, L], F16)
    nc.scalar.activation(xsq[:, :XCUT], x_all[:, :XCUT], ACT.Square)
    nc.scalar.activation(xsq[:, XCUT:], x_all[:, XCUT:], ACT.Square)

    # ---- shapelet prep: st4 (phase-major -2*sh^T | sq_s) and lhsT tiles ----
    # All prep DMAs go through the Activation DGE queue so the Sync queue is
    # free to start the main-loop im2col reads as soon as deps resolve.
    prep = ctx.enter_context(tc.tile_pool(name="prep", bufs=1))
    with tc.tile_pool(name="prep_ps", bufs=1, space="PSUM") as prep_ps:
        pt = prep_ps.tile([53, C * K], F32, tag="pt")
        st4 = prep.tile([53, C * K], BF16, tag="st4")
        for c in range(C):
            sh_pad = prep.tile([K, 52], F32, tag=f"sh_pad{c}")
            nc.vector.memset(sh_pad[:, S:], 0.0)
            nc.scalar.dma_start(sh_pad[:, :S], sh_ap[c])
            # sh_m2p cols q = 13*(s%4) + s//4 hold -2*sh[:, s]; col 52 = sq_s
            sh_m2p = prep.tile([K, 53], F32, tag=f"sh_m2p{c}")
            perm_dst = bass.AP(
                sh_m2p.tensor, sh_m2p.offset, [[sh_m2p.ap[0][0], K], [13, PH], [1, 13]]
            )
            perm_src = bass.AP(
                sh_pad.tensor, sh_pad.offset, [[sh_pad.ap[0][0], K], [1, PH], [PH, 13]]
            )
            nc.scalar.activation(perm_dst, perm_src, ACT.Copy, scale=-2.0)
            sh_sq = prep.tile([K, S], F32, tag=f"sh_sq{c}")
            nc.scalar.activation(
                sh_sq[:], sh_pad[:, :S], ACT.Square,
                accum_out=sh_m2p[:, 52:53],
            )
            nc.tensor.transpose(pt[:, c * K : (c + 1) * K], sh_m2p[:], ident[:])
        nc.scalar.activation(st4[:], pt[:], ACT.Copy)
        # touch Sqrt now so its ACT table loads during prep, not at the
        # first main-loop tile
        sq_warm = prep.tile([1, 1], F32, tag="sq_warm")
        nc.scalar.activation(sq_warm[:], ident[0:1, 0:1], ACT.Sqrt)
        for j in range(PH):
            lt = lhsT[j]
            nc.gpsimd.memset(lt[:], 0.0)
            for p in range(PH):
                d = (p - j) % PH
                a0 = 0 if p >= j else 1
                eng = nc.scalar if p % 2 == 0 else nc.sync
                eng.dma_start(
                    lt[p * RXP + a0 : p * RXP + a0 + 13, :],
                    st4[d * 13 : d * 13 + 13, :],
                )
            # sqw coefficient: phase j row
            nc.scalar.dma_start(lt[NXROW + j : NXROW + j + 1, :], ones_blk[:])
            # sq_s row (pairs with the ones block of sqw4)
            nc.sync.dma_start(lt[CONTRACT - 1 : CONTRACT, :], st4[52:53, :])

    # ---- x prep: deinterleave, squares tree, spill to DRAM ----
    if True:
        # x4[bc, p*BLK + z] = x[bc, PH*z + p] for z < LQ, 0.0 in the pad tail
        x4 = xp.tile([BC, PH * BLK], BF16)
        pad = bass.AP(
            x4.tensor,
            x4[0:1, LQ : LQ + 1].offset,
            [[x4.ap[0][0], BC], [BLK, PH], [1, BLK - LQ]],
        )
        nc.gpsimd.memset(pad, 0.0)
        x_deint = bass.AP(
            x_all.tensor, x_all.offset, [[x_all.ap[0][0], BC], [1, PH], [PH, LQ]]
        )
        x4_dst = bass.AP(
            x4.tensor, x4.offset, [[x4.ap[0][0], BC], [BLK, PH], [1, LQ]]
        )
        s2 = xp.tile([BC, L], F16)
        s4 = xp.tile([BC, L], F16)
        s8 = xp.tile([BC, L], F16)
        s16 = xp.tile([BC, L], F16)
        s32 = xp.tile([BC, L], F16)
        s48 = xp.tile([BC, L], F16)
        sqw = xp.tile([BC, W + 1], F16)
        nc.gpsimd.memset(sqw[:, W : W + 1], SQWPAD)
        levels = [
            (s2, xsq, xsq, 1, L - 1, 1024),
            (s4, s2, s2, 2, L - 3, 1022),
            (s8, s4, s4, 4, L - 7, 1018),
            (s16, s8, s8, 8, L - 15, 1010),
            (s32, s16, s16, 16, L - 31, 994),
            (s48, s32, s16, 32, L - 47, 962),
            (sqw, s48, s2, 48, W, 962),
        ]
        for dst, a, b, h, v, e in levels:  # early chain: only needs xsq[:XCUT]
            nc.vector.tensor_add(dst[:, :e], a[:, :e], b[:, h : h + e])
        nc.vector.tensor_copy(x4_dst, x_deint)
        nc.sync.dma_start(xd_ap, x4[:])
        for dst, a, b, h, v, e in levels:  # late chain
            nc.vector.tensor_add(dst[:, e:v], a[:, e:v], b[:, e + h : v + h])

        sqw4 = xp.tile([BC, (PH + 1) * T], BF16)
        nc.gpsimd.memset(sqw4[:], SQWPAD)
        ones_t = bass.AP(
            sqw4.tensor,
            sqw4[0:1, PH * T : PH * T + 1].offset,
            [[sqw4.ap[0][0], BC], [1, T]],
        )
        nc.gpsimd.memset(ones_t, 1.0)
        nq = 500  # deinterleave reads sqw cols j + PH*t, t < 500 (max 1999)
        sq_src = bass.AP(
            sqw.tensor, sqw.offset, [[sqw.ap[0][0], BC], [1, PH], [PH, nq]]
        )
        sq_dst = bass.AP(
            sqw4.tensor, sqw4.offset, [[sqw4.ap[0][0], BC], [T, PH], [1, nq]]
        )
        nc.vector.tensor_copy(sq_dst, sq_src)
        nc.gpsimd.dma_start(sqd_ap, sqw4[:])

    # ---- main loop ----
    rhsp = ctx.enter_context(tc.tile_pool(name="rhs", bufs=6))
    psum = ctx.enter_context(tc.tile_pool(name="mm", bufs=2, space="PSUM"))
    distp = ctx.enter_context(tc.tile_pool(name="dist", bufs=3))

    for b in range(BPC):
        dist = []
        for c in range(C):
            bc = b * C + c
            rhs = rhsp.tile([CONTRACT, T], BF16, tag="rhs")
            # x im2col rows from DRAM: row p*RXP+a = x4[bc, p*BLK + a + t]
            nc.sync.dma_start(
                rhs[:NXROW, :],
                bass.AP(
                    xd_ap.tensor,
                    bc * PH * BLK,
                    [[PH * BLK, 1], [BLK, PH], [1, RXP], [1, T]],
                ),
            )
            # sqw phase rows + ones row from DRAM (issued from GpSimd's queue
            # to keep the Sync queue for the big im2col reads)
            nc.gpsimd.dma_start(
                rhs[NXROW:CONTRACT, :],
                bass.AP(
                    sqd_ap.tensor,
                    bc * (PH + 1) * T,
                    [[(PH + 1) * T, 1], [T, PH + 1], [1, T]],
                ),
            )
            d2 = psum.tile([K, PH * T], F32, tag="d2")
            for j in range(PH):
                nc.tensor.matmul(
                    d2[:, j * T : (j + 1) * T],
                    lhsT[j][:, c * K : (c + 1) * K],
                    rhs[:],
                    start=True,
                    stop=True,
                )
            dt_ = distp.tile([K, PH * T], F16, tag=f"dist{c}", name=f"dist{c}")
            nc.scalar.activation(dt_[:], d2[:], ACT.Sqrt)
            dist.append(dt_)
        t01 = distp.tile([K, PH * T], F16, tag="t01")
        nc.vector.tensor_add(t01[:], dist[0][:], dist[1][:])
        scr = distp.tile([K, PH * T], F16, tag="scr")
        nc.vector.tensor_add(scr[:], t01[:], dist[2][:])
        nc.vector.tensor_reduce(
            res[:, b : b + 1], scr[:], axis=AXIS.X, op=ALU.min
        )

    # ---- store result as (K, BPC); the host unshard transposes ----
    nc.sync.dma_start(out_ap, res[:])


def _build():
    nc = bacc.Bacc(
        "TRN2", target_bir_lowering=False, debug=False, num_devices=NCORES
    )
    x = nc.dram_tensor("x", [BPC, C, L], F32, kind="ExternalInput").ap()
    sh = nc.dram_tensor("sh", [C, K, S], F32, kind="ExternalInput").ap()
    out = nc.dram_tensor("out", [K, BPC], F32, kind="ExternalOutput").ap()
    xd = nc.dram_tensor("xs4", [BC, PH * BLK], BF16, kind="Internal").ap()
    sqd = nc.dram_tensor("sqw4", [BC, (PH + 1) * T], BF16, kind="Internal").ap()
    with tile.TileContext(nc) as tc, ExitStack() as ctx:
        _body(ctx, tc, out, x, sh, xd, sqd)
    nc.compile()
    return nc


def kernel(x, shapelets, trace=False):
    global LAST_RESULTS
    x = np.ascontiguousarray(np.asarray(x, dtype=np.float32))
    shapelets = np.ascontiguousarray(np.asarray(shapelets, dtype=np.float32))
    nc = _build()
    in_maps = [
        {"x": x[i * BPC : (i + 1) * BPC], "sh": shapelets} for i in range(NCORES)
    ]
    results = run_bass_kernel_spmd(
        nc, in_maps, core_ids=list(range(NCORES)), trace=trace
    )
    LAST_RESULTS = results
    out = np.concatenate(
        [results.results[i]["out"].T for i in range(NCORES)], axis=0
    )
    return np.ascontiguousarray(out).reshape(B, 1, K)


# revision 34
# speedup vs baseline: 1.1803x; 1.0051x over previous
"""Trainium2 Bass kernel for MinEuclideanDistBlock.

Math (per batch b):
  d2[c,w,k] = ||x[b,c,w:w+S]||^2 + ||sh[c,k]||^2 - 2 <x[b,c,w:w+S], sh[c,k]>
  out[b,k]  = min_w  sum_c sqrt(d2[c,w,k])

Kernel strategy (per core, data-parallel over batch: 16 of 128 batches):
  - One matmul per (b,c,phase) produces d2 directly in PSUM via an
    augmented 61-row contraction: 56 rows of a stride-4 im2col of x
    (phase-major groups of 14), 4 rows of the phase-split sliding
    ||window||^2 (computed once by a log-doubling shift-add tree), and a
    ones row paired with sq_s.
  - The stride-4 im2col cuts duplication ~4x vs stride-1; its source
    (the phase-deinterleaved x) is round-tripped through HBM so the
    per-(b,c) im2col read is a DRAM->SBUF transfer of contiguous 1KB
    lines (SBUF->SBUF overlapping-line DMA measured ~3GB/s/engine on
    the previous version; DRAM reads are the fast standard pattern).
  - d2 is phase-major [K, 4*512]; column order is irrelevant under the
    final min over windows, so each matmul writes one contiguous PSUM
    bank.  Out-of-range windows see a +50000 sq_w pad and become huge
    valid distances, so no masking is needed downstream.
  - dist = sqrt(d2): one scalar-engine activation per (b,c); this is the
    critical engine (~2us per (b,c) at 1 elem/cycle/lane).
  - channel sum + min over windows: two tensor_adds plus a min
    tensor_reduce per batch on the vector engine (tensor_tensor_reduce
    would fuse these but crashes the runtime on this stack).
  - matmul operands are bfloat16, sqw tree and dist tiles float16
    (end-to-end rel err ~2.4e-3, far inside the 2e-2 gate).
  - the PE clock is pinned at 1.2GHz in this environment (a sustained
    dummy-matmul burst does not flip the HAM clock gate), so the PE
    stream time is the binding engine alongside the scalar-engine sqrt.
"""

import numpy as np
from contextlib import ExitStack

import concourse.bass as bass
import concourse.bacc as bacc
import concourse.mybir as mybir
import concourse.tile as tile
from concourse import masks
from concourse.bass_utils import run_bass_kernel_spmd

B, C, L = 128, 3, 2048
K, S = 128, 50
W = L - S + 1  # 1999
NCORES = 8
BPC = B // NCORES  # batches per core
BC = BPC * C  # x rows per core

PH = 4  # im2col stride / number of window phases
T = 512  # columns per phase (= one PSUM bank of fp32)
RXP = 14  # im2col rows per phase group
NXROW = PH * RXP  # 56 x rows
CONTRACT = NXROW + PH + 1  # + sqw phase rows + ones(sq_s) row = 61
LQ = L // PH  # real elements per phase block (512)
BLK = LQ + RXP  # x4 phase-block pitch (526); max read 13+511=524 < 526
SQWPAD = 50000.0  # sq_w pad: any window w >= W gets d2 ~ 5e4 -> dist ~ 224

F32 = mybir.dt.float32
F16 = mybir.dt.float16
BF16 = mybir.dt.bfloat16
ACT = mybir.ActivationFunctionType
ALU = mybir.AluOpType
AXIS = mybir.AxisListType

LAST_RESULTS = None  # BassKernelResults of the last run (for test harness)


def _body(ctx, tc, out_ap, x_ap, sh_ap, xd_ap, sqd_ap):
    nc = tc.nc

    const = ctx.enter_context(tc.tile_pool(name="const", bufs=1))
    ident = const.tile([128, 128], F32)
    masks.make_identity(nc, ident[:])
    ones_blk = const.tile([1, C * K], BF16)
    nc.vector.memset(ones_blk[:], 1.0)
    # one stationary tile per phase, all 3 channels side by side in the free dim
    lhsT = [
        const.tile([CONTRACT, C * K], BF16, tag=f"lhsT{j}", name=f"lhsT{j}")
        for j in range(PH)
    ]
    persist = ctx.enter_context(tc.tile_pool(name="persist", bufs=1))
    res = persist.tile([K, BPC], F32)

    # ---- x load + x^2 first: ACT is idle this early, and the DVE tree
    # depends on xsq, so this pulls the whole x chain forward ----
    xp = ctx.enter_context(tc.tile_pool(name="xprep", bufs=1))
    x_all = xp.tile([BC, L], F32)
    x_rows = x_ap.rearrange("b c l -> (b c) l")
    for lo, hi, eng in (
        (0, 688, nc.sync),
        (688, 1376, nc.scalar),
        (1376, L, nc.gpsimd),
    ):
        eng.dma_start(x_all[:, lo:hi], x_rows[:, lo:hi])
    # x^2 in two column pieces so the early tree chain can start before the
    # whole x transfer lands
    XCUT = 1032
    xsq = xp.tile([BC, L], F16)
    nc.scalar.activation(xsq[:, :XCUT], x_all[:, :XCUT], ACT.Square)
    nc.scalar.activation(xsq[:, XCUT:], x_all[:, XCUT:], ACT.Square)

    # ---- shapelet prep: st4 (phase-major -2*sh^T | sq_s) and lhsT tiles ----
    # All prep DMAs go through the Activation DGE queue so the Sync queue is
    # free to start the main-loop im2col reads as soon as deps resolve.
    prep = ctx.enter_context(tc.tile_pool(name="prep", bufs=1))
    with tc.tile_pool(name="prep_ps", bufs=1, space="PSUM") as prep_ps:
        pt = prep_ps.tile([53, C * K], F32, tag="pt")
        st4 = prep.tile([53, C * K], BF16, tag="st4")
        for c in range(C):
            sh_pad = prep.tile([K, 52], F32, tag=f"sh_pad{c}")
            nc.vector.memset(sh_pad[:, S:], 0.0)
            nc.scalar.dma_start(sh_pad[:, :S], sh_ap[c])
            # sh_m2p cols q = 13*(s%4) + s//4 hold -2*sh[:, s]; col 52 = sq_s
            sh_m2p = prep.tile([K, 53], F32, tag=f"sh_m2p{c}")
            perm_dst = bass.AP(
                sh_m2p.tensor, sh_m2p.offset, [[sh_m2p.ap[0][0], K], [13, PH], [1, 13]]
            )
            perm_src = bass.AP(
                sh_pad.tensor, sh_pad.offset, [[sh_pad.ap[0][0], K], [1, PH], [PH, 13]]
            )
            nc.scalar.activation(perm_dst, perm_src, ACT.Copy, scale=-2.0)
            sh_sq = prep.tile([K, S], F32, tag=f"sh_sq{c}")
            nc.scalar.activation(
                sh_sq[:], sh_pad[:, :S], ACT.Square,
                accum_out=sh_m2p[:, 52:53],
            )
            nc.tensor.transpose(pt[:, c * K : (c + 1) * K], sh_m2p[:], ident[:])
        nc.scalar.activation(st4[:], pt[:], ACT.Copy)
        # touch Sqrt now so its ACT table loads during prep, not at the
        # first main-loop tile
        sq_warm = prep.tile([1, 1], F32, tag="sq_warm")
        nc.scalar.activation(sq_warm[:], ident[0:1, 0:1], ACT.Sqrt)
        for j in range(PH):
            lt = lhsT[j]
            nc.gpsimd.memset(lt[:], 0.0)
            for p in range(PH):
                d = (p - j) % PH
                a0 = 0 if p >= j else 1
                eng = nc.scalar if p % 2 == 0 else nc.sync
                eng.dma_start(
                    lt[p * RXP + a0 : p * RXP + a0 + 13, :],
                    st4[d * 13 : d * 13 + 13, :],
                )
            # sqw coefficient: phase j row
            nc.scalar.dma_start(lt[NXROW + j : NXROW + j + 1, :], ones_blk[:])
            # sq_s row (pairs with the ones block of sqw4)
            nc.sync.dma_start(lt[CONTRACT - 1 : CONTRACT, :], st4[52:53, :])

    # ---- x prep: deinterleave, squares tree, spill to DRAM ----
    if True:
        # x4[bc, p*BLK + z] = x[bc, PH*z + p] for z < LQ, 0.0 in the pad tail
        x4 = xp.tile([BC, PH * BLK], BF16)
        pad = bass.AP(
            x4.tensor,
            x4[0:1, LQ : LQ + 1].offset,
            [[x4.ap[0][0], BC], [BLK, PH], [1, BLK - LQ]],
        )
        nc.gpsimd.memset(pad, 0.0)
        x_deint = bass.AP(
            x_all.tensor, x_all.offset, [[x_all.ap[0][0], BC], [1, PH], [PH, LQ]]
        )
        x4_dst = bass.AP(
            x4.tensor, x4.offset, [[x4.ap[0][0], BC], [BLK, PH], [1, LQ]]
        )
        s2 = xp.tile([BC, L], F16)
        s4 = xp.tile([BC, L], F16)
        s8 = xp.tile([BC, L], F16)
        s16 = xp.tile([BC, L], F16)
        s32 = xp.tile([BC, L], F16)
        s48 = xp.tile([BC, L], F16)
        sqw = xp.tile([BC, W + 1], F16)
        nc.gpsimd.memset(sqw[:, W : W + 1], SQWPAD)
        levels = [
            (s2, xsq, xsq, 1, L - 1, 1024),
            (s4, s2, s2, 2, L - 3, 1022),
            (s8, s4, s4, 4, L - 7, 1018),
            (s16, s8, s8, 8, L - 15, 1010),
            (s32, s16, s16, 16, L - 31, 994),
            (s48, s32, s16, 32, L - 47, 962),
            (sqw, s48, s2, 48, W, 962),
        ]
        for dst, a, b, h, v, e in levels:  # early chain: only needs xsq[:XCUT]
            nc.vector.tensor_add(dst[:, :e], a[:, :e], b[:, h : h + e])
        nc.vector.tensor_copy(x4_dst, x_deint)
        nc.sync.dma_start(xd_ap, x4[:])
        for dst, a, b, h, v, e in levels:  # late chain
            nc.vector.tensor_add(dst[:, e:v], a[:, e:v], b[:, e + h : v + h])

        sqw4 = xp.tile([BC, (PH + 1) * T], BF16)
        nc.gpsimd.memset(sqw4[:], SQWPAD)
        ones_t = bass.AP(
            sqw4.tensor,
            sqw4[0:1, PH * T : PH * T + 1].offset,
            [[sqw4.ap[0][0], BC], [1, T]],
        )
        nc.gpsimd.memset(ones_t, 1.0)
        nq = 500  # deinterleave reads sqw cols j + PH*t, t < 500 (max 1999)
        sq_src = bass.AP(
            sqw.tensor, sqw.offset, [[sqw.ap[0][0], BC], [1, PH], [PH, nq]]
        )
        sq_dst = bass.AP(
            sqw4.tensor, sqw4.offset, [[sqw4.ap[0][0], BC], [T, PH], [1, nq]]
        )
        nc.vector.tensor_copy(sq_dst, sq_src)
        nc.gpsimd.dma_start(sqd_ap, sqw4[:])

    # ---- main loop ----
    rhsp = ctx.enter_context(tc.tile_pool(name="rhs", bufs=6))
    psum = ctx.enter_context(tc.tile_pool(name="mm", bufs=2, space="PSUM"))
    distp = ctx.enter_context(tc.tile_pool(name="dist", bufs=3))

    for b in range(BPC):
        dist = []
        for c in range(C):
            bc = b * C + c
            rhs = rhsp.tile([CONTRACT, T], BF16, tag="rhs")
            # x im2col rows from DRAM: row p*RXP+a = x4[bc, p*BLK + a + t]
            nc.sync.dma_start(
                rhs[:NXROW, :],
                bass.AP(
                    xd_ap.tensor,
                    bc * PH * BLK,
                    [[PH * BLK, 1], [BLK, PH], [1, RXP], [1, T]],
                ),
            )
            # sqw phase rows + ones row from DRAM (issued from GpSimd's queue
            # to keep the Sync queue for the big im2col reads)
            nc.gpsimd.dma_start(
                rhs[NXROW:CONTRACT, :],
                bass.AP(
                    sqd_ap.tensor,
                    bc * (PH + 1) * T,
                    [[(PH + 1) * T, 1], [T, PH + 1], [1, T]],
                ),
            )
            d2 = psum.tile([K, PH * T], F32, tag="d2")
            for j in range(PH):
                nc.tensor.matmul(
                    d2[:, j * T : (j + 1) * T],
                    lhsT[j][:, c * K : (c + 1) * K],
                    rhs[:],
                    start=True,
                    stop=True,
                )
            dt_ = distp.tile([K, PH * T], F16, tag=f"dist{c}", name=f"dist{c}")
            nc.scalar.activation(dt_[:], d2[:], ACT.Sqrt)
            dist.append(dt_)
        t01 = distp.tile([K, PH * T], F16, tag="t01")
        nc.vector.tensor_add(t01[:], dist[0][:], dist[1][:])
        scr = distp.tile([K, PH * T], F16, tag="scr")
        nc.vector.tensor_add(scr[:], t01[:], dist[2][:])
        nc.vector.tensor_reduce(
            res[:, b : b + 1], scr[:], axis=AXIS.X, op=ALU.min
        )

    # ---- store result as (K, BPC); the host unshard transposes ----
    nc.sync.dma_start(out_ap, res[:])


def _build():
    nc = bacc.Bacc(
        "TRN2", target_bir_lowering=False, debug=False, num_devices=NCORES
    )
    x = nc.dram_tensor("x", [BPC, C, L], F32, kind="ExternalInput").ap()
    sh = nc.dram_tensor("sh", [C, K, S], F32, kind="ExternalInput").ap()
    out = nc.dram_tensor("out", [K, BPC], F32, kind="ExternalOutput").ap()
    xd = nc.dram_tensor("xs4", [BC, PH * BLK], BF16, kind="Internal").ap()
    sqd = nc.dram_tensor("sqw4", [BC, (PH + 1) * T], BF16, kind="Internal").ap()
    with tile.TileContext(nc) as tc, ExitStack() as ctx:
        _body(ctx, tc, out, x, sh, xd, sqd)
    nc.compile()
    return nc


def kernel(x, shapelets, trace=False):
    global LAST_RESULTS
    x = np.ascontiguousarray(np.asarray(x, dtype=np.float32))
    shapelets = np.ascontiguousarray(np.asarray(shapelets, dtype=np.float32))
    nc = _build()
    in_maps = [
        {"x": x[i * BPC : (i + 1) * BPC], "sh": shapelets} for i in range(NCORES)
    ]
    results = run_bass_kernel_spmd(
        nc, in_maps, core_ids=list(range(NCORES)), trace=trace
    )
    LAST_RESULTS = results
    out = np.concatenate(
        [results.results[i]["out"].T for i in range(NCORES)], axis=0
    )
    return np.ascontiguousarray(out).reshape(B, 1, K)
